# revision 2
# baseline (speedup 1.0000x reference)
"""3-layer GCN (100k nodes, 1.6M edges, 128->128->128->40) on 8 trn2 cores.

Self-contained harness kernel: kernel(**inputs) takes the FULL unsharded
inputs and returns the FULL [100000, 40] float32 output.

Strategy (1D node partition, edges sharded by dst, per the standard GCN
distribution):
  - nodes split contiguously across the 8 cores (12500 each, padded 12544);
    edges assigned to the core owning their dst.
  - per layer, each core computes the dense transform hp = (h @ W) * ns[row]
    for its own rows on the PE (fp32), stores rows as [bf16 hi | bf16 lo]
    (hi/lo split keeps f32-grade accuracy with 1-cycle bf16 matmuls), then an
    AllGather replicates the 100352-row table to every core.
  - aggregation: edges are grouped by 128-wide dst windows; per 128-edge
    chunk a dma_gather pulls the src rows (512B descriptors) and a one-hot
    S matrix (built on DVE from compile-time dst metadata via a broadcast
    is_equal against an iota tile) routes them into a PSUM accumulator via
    two bf16 matmuls (hi+lo).  Window epilogue applies nd / bias / relu on
    DVE+ACT, and the transposed result feeds the next layer's transform as
    the stationary matmul operand directly (no transposes anywhere).
  - gather indices are int16 (table bucketed 4x25088 rows); per-(window,
    bucket) runs are padded to 128 and unioned across cores so all 8 cores
    share one SPMD program.
"""
import sys
sys.path.insert(0, '/opt/trn_rl_repo')

import math
import numpy as np

import concourse.bass as bass
import concourse.bacc as bacc
import concourse.tile as tile
import concourse.mybir as mybir
from concourse.bass_utils import run_bass_kernel_spmd

f32 = mybir.dt.float32
bf16 = mybir.dt.bfloat16
i16 = mybir.dt.int16

NC = 8
GW = 4  # windows per gather group


def _preprocess(src, dst, n_nodes):
    src = np.asarray(src).astype(np.int64)
    dst = np.asarray(dst).astype(np.int64)
    N = n_nodes
    assert N % NC == 0
    shard = N // NC
    NW = (shard + 127) // 128
    padshard = NW * 128
    NPAD = NC * padshard
    NBUK = max(1, math.ceil(NPAD / 32768))
    buksz = math.ceil(NPAD / NBUK / 128) * 128

    outdeg = np.bincount(src, minlength=N)
    indeg = np.bincount(dst, minlength=N)
    ns = (1.0 / np.sqrt(np.maximum(outdeg, 1))).astype(np.float32)
    nd = (1.0 / np.sqrt(np.maximum(indeg, 1))).astype(np.float32)

    srcg = (src // shard) * padshard + (src % shard)
    ecore = dst // shard

    cores = []
    counts = np.zeros((NC, NW, NBUK), dtype=np.int64)
    for c in range(NC):
        m = ecore == c
        es = srcg[m]
        ld = dst[m] - c * shard
        w = ld >> 7
        slot = ld & 127
        b = es // buksz
        reb = es - b * buksz
        order = np.lexsort((es, b, w))
        w, b, slot, reb = w[order], b[order], slot[order], reb[order]
        key = w * NBUK + b
        cnt = np.bincount(key, minlength=NW * NBUK).reshape(NW, NBUK)
        counts[c] = cnt
        cores.append((w, b, slot, reb, key, cnt))

    C_wb = (counts.max(axis=0) + 127) // 128
    C_wb[:, 0] = np.maximum(C_wb[:, 0], 1)
    C_w = C_wb.sum(axis=1)
    TOTCOL = int(C_w.sum())
    TOTSLOT = TOTCOL * 128

    colbase_wb = np.zeros((NW, NBUK), dtype=np.int64)
    acc = 0
    for w_ in range(NW):
        for b_ in range(NBUK):
            colbase_wb[w_, b_] = acc
            acc += C_wb[w_, b_]
    colbase_w = colbase_wb[:, 0]

    groups = []
    idx_off = 0
    NG = (NW + GW - 1) // GW
    for g in range(NG):
        ws = list(range(g * GW, min((g + 1) * GW, NW)))
        calls = []
        slabcol = 0
        for b_ in range(NBUK):
            n_cols = int(C_wb[ws[0]:ws[-1] + 1, b_].sum())
            if n_cols == 0:
                calls.append(None)
                continue
            n_idx = n_cols * 128
            calls.append((b_, idx_off, n_idx, slabcol))
            idx_off += n_idx
            slabcol += n_cols
        groups.append((ws, calls, slabcol))
    assert idx_off == TOTSLOT

    gcol_of = []
    for g, (ws, calls, _) in enumerate(groups):
        for wi, w_ in enumerate(ws):
            lst = []
            for b_ in range(NBUK):
                if calls[b_] is None:
                    continue
                base = calls[b_][3] + int(C_wb[ws[0]:w_, b_].sum())
                for j in range(int(C_wb[w_, b_])):
                    lst.append(base + j)
            gcol_of.append(lst)

    per_core = []
    for c in range(NC):
        w, b, slot, reb, key, cnt = cores[c]
        run_start = np.zeros(NW * NBUK, dtype=np.int64)
        run_start[1:] = np.cumsum(np.bincount(key, minlength=NW * NBUK))[:-1]
        in_run = np.arange(len(key)) - run_start[key]
        base_flat = colbase_wb.reshape(-1) * 128
        pos = base_flat[key] + in_run

        dstl_flat = np.full(TOTSLOT, -1.0, dtype=np.float32)
        dstl_flat[pos] = slot.astype(np.float32)
        idx_flat_winmajor = np.zeros(TOTSLOT, dtype=np.int16)
        idx_flat_winmajor[pos] = reb.astype(np.int16)

        idx_flat = np.zeros(TOTSLOT, dtype=np.int16)
        off = 0
        for g, (ws, calls, _) in enumerate(groups):
            for b_ in range(NBUK):
                if calls[b_] is None:
                    continue
                for w_ in ws:
                    cwb = int(C_wb[w_, b_])
                    if cwb == 0:
                        continue
                    s0 = int(colbase_wb[w_, b_]) * 128
                    n = cwb * 128
                    idx_flat[off:off + n] = idx_flat_winmajor[s0:s0 + n]
                    off += n

        dstl2d = dstl_flat.reshape(TOTCOL, 128).T.copy()
        idx2d = np.tile(idx_flat.reshape(TOTCOL * 8, 16).T, (8, 1)).copy()

        ns_sh = np.zeros(padshard, dtype=np.float32)
        nd_sh = np.zeros(padshard, dtype=np.float32)
        ns_sh[:shard] = ns[c * shard:(c + 1) * shard]
        nd_sh[:shard] = nd[c * shard:(c + 1) * shard]
        nscol = ns_sh.reshape(NW, 128).T.copy()
        ndcol = nd_sh.reshape(NW, 128).T.copy()
        ndrep = np.tile(nd_sh[None, :], (128, 1))

        per_core.append(dict(dstl=dstl2d, idx=idx2d, nscol=nscol, ndcol=ndcol,
                             ndrep=ndrep))

    struct = dict(N=N, shard=shard, NW=NW, padshard=padshard, NPAD=NPAD,
                  NBUK=NBUK, buksz=buksz, C_w=C_w, TOTCOL=TOTCOL,
                  TOTSLOT=TOTSLOT, colbase_w=colbase_w, groups=groups,
                  gcol_of=gcol_of)
    return struct, per_core


def _build_program(st, f_cls):
    NW, padshard, NPAD = st['NW'], st['padshard'], st['NPAD']
    NBUK, buksz = st['NBUK'], st['buksz']
    C_w, TOTCOL, TOTSLOT = st['C_w'], st['TOTCOL'], st['TOTSLOT']
    colbase_w, groups, gcol_of = st['colbase_w'], st['groups'], st['gcol_of']
    shard = st['shard']
    fcp = 64 * ((f_cls + 63) // 64)

    nc = bacc.Bacc(None, target_bir_lowering=False)

    featT_d = nc.dram_tensor("featT", [128, padshard], f32, kind="ExternalInput")
    idx_d = nc.dram_tensor("idx16", [128, TOTSLOT // 16], i16, kind="ExternalInput")
    dstl_d = nc.dram_tensor("dstl", [128, TOTCOL], bf16, kind="ExternalInput")
    iota_d = nc.dram_tensor("iota", [128, 128], bf16, kind="ExternalInput")
    ndrep_d = nc.dram_tensor("ndrep", [128, padshard], f32, kind="ExternalInput")
    nscol_d = nc.dram_tensor("nscol", [128, NW], f32, kind="ExternalInput")
    ndcol_d = nc.dram_tensor("ndcol", [128, NW], f32, kind="ExternalInput")
    W0_d = nc.dram_tensor("W0", [128, 128], f32, kind="ExternalInput")
    W1_d = nc.dram_tensor("W1", [128, 128], f32, kind="ExternalInput")
    W2_d = nc.dram_tensor("W2p", [128, fcp], f32, kind="ExternalInput")
    b0_d = nc.dram_tensor("b0c", [128, 1], f32, kind="ExternalInput")
    b1_d = nc.dram_tensor("b1c", [128, 1], f32, kind="ExternalInput")
    b2_d = nc.dram_tensor("b2rep", [128, fcp], f32, kind="ExternalInput")
    out_d = nc.dram_tensor("out", [shard, f_cls], f32, kind="ExternalOutput")

    hp0_own = nc.dram_tensor("hp0_own", [padshard, 256], bf16)
    hp1_own = nc.dram_tensor("hp1_own", [padshard, 256], bf16)
    hp2_own = nc.dram_tensor("hp2_own", [padshard, 2 * fcp], bf16)
    hp0_full = nc.dram_tensor("hp0_full", [NPAD, 256], bf16, addr_space="Shared")
    hp1_full = nc.dram_tensor("hp1_full", [NPAD, 256], bf16, addr_space="Shared")
    hp2_full = nc.dram_tensor("hp2_full", [NPAD, 2 * fcp], bf16, addr_space="Shared")

    rg = [list(range(NC))]

    with tile.TileContext(nc) as tc:
        with (
            tc.tile_pool(name="const", bufs=1) as cpool,
            tc.tile_pool(name="gpool", bufs=2) as gpool,
            tc.tile_pool(name="spool", bufs=3) as spool,
            tc.tile_pool(name="wpool", bufs=3) as wpool,
            tc.tile_pool(name="xpool", bufs=3) as xpool,
            tc.tile_pool(name="ipool", bufs=2) as ipool,
            tc.tile_pool(name="psA", bufs=2, space="PSUM") as psA,
            tc.tile_pool(name="psC", bufs=4, space="PSUM") as psC,
        ):
            sW0 = cpool.tile([128, 128], f32); nc.sync.dma_start(sW0[:], W0_d[:])
            sW1 = cpool.tile([128, 128], f32); nc.sync.dma_start(sW1[:], W1_d[:])
            sW2 = cpool.tile([128, fcp], f32); nc.sync.dma_start(sW2[:], W2_d[:])
            sb0 = cpool.tile([128, 1], f32); nc.sync.dma_start(sb0[:], b0_d[:])
            sb1 = cpool.tile([128, 1], f32); nc.sync.dma_start(sb1[:], b1_d[:])
            sb2 = cpool.tile([128, fcp], f32); nc.sync.dma_start(sb2[:], b2_d[:])
            siota = cpool.tile([128, 128], bf16); nc.sync.dma_start(siota[:], iota_d[:])
            sdstl = cpool.tile([128, TOTCOL], bf16); nc.sync.dma_start(sdstl[:], dstl_d[:])
            snscol = cpool.tile([128, NW], f32); nc.sync.dma_start(snscol[:], nscol_d[:])
            sndcol = cpool.tile([128, NW], f32); nc.sync.dma_start(sndcol[:], ndcol_d[:])

            def transform_split(w, lhsT_ap, sW, fo, hp_own):
                ps2 = psA.tile([128, fo], f32)
                nc.tensor.matmul(ps2[:], lhsT_ap, sW, start=True, stop=True)
                tns = xpool.tile([128, fo], f32, tag="tns")
                nc.vector.tensor_scalar_mul(tns[:], ps2[:], snscol[:, w:w + 1])
                hp = xpool.tile([128, 2 * fo], bf16, tag="hp")
                nc.scalar.activation(hp[:, 0:fo], tns[:],
                                     mybir.ActivationFunctionType.Copy)
                hif = xpool.tile([128, fo], f32, tag="hif")
                nc.scalar.activation(hif[:], hp[:, 0:fo],
                                     mybir.ActivationFunctionType.Copy)
                nc.vector.tensor_tensor(hp[:, fo:2 * fo], tns[:], hif[:],
                                        mybir.AluOpType.subtract)
                nc.sync.dma_start(hp_own[w * 128:(w + 1) * 128, :], hp[:])

            for w in range(NW):
                ft = wpool.tile([128, 128], f32, tag="ft")
                nc.sync.dma_start(ft[:], featT_d[:, w * 128:(w + 1) * 128])
                transform_split(w, ft[:], sW0[:], 128, hp0_own)
            nc.gpsimd.collective_compute("AllGather", mybir.AluOpType.bypass, rg,
                                         ins=[hp0_own[:, :]], outs=[hp0_full[:, :]])

            def agg_layer(hp_full, elem, layer):
                for (ws, calls, C_g) in groups:
                    G = gpool.tile([128, C_g, 2 * elem], bf16, tag="G")
                    idxs = ipool.tile([128, max(1, C_g) * 8], i16, tag="idx")
                    for call in calls:
                        if call is None:
                            continue
                        b_, off, n_idx, slabcol = call
                        nc.sync.dma_start(
                            idxs[:, slabcol * 8:slabcol * 8 + n_idx // 16],
                            idx_d[:, off // 16:(off + n_idx) // 16])
                        rows = min(buksz, NPAD - b_ * buksz)
                        nc.gpsimd.dma_gather(
                            out_ap=G[:, slabcol:slabcol + n_idx // 128, :],
                            in_ap=hp_full[b_ * buksz:b_ * buksz + rows, :],
                            idxs_ap=idxs[:16, slabcol * 8:slabcol * 8 + n_idx // 16],
                            num_idxs=n_idx,
                            num_idxs_reg=n_idx,
                            elem_size=2 * elem,
                            single_packet=False,
                        )
                    for w in ws:
                        cw = int(C_w[w])
                        cb = int(colbase_w[w])
                        S = spool.tile([128, cw * 128], bf16, tag="S")
                        in0 = sdstl[:, cb:cb + cw].unsqueeze(2).broadcast_to([128, cw, 128])
                        in1 = siota[:, :].unsqueeze(1).broadcast_to([128, cw, 128])
                        nc.vector.tensor_tensor(
                            S[:, :].rearrange("p (c x) -> p c x", x=128),
                            in0, in1, mybir.AluOpType.is_equal)
                        if layer < 2:
                            ps = psC.tile([128, 128], f32, tag="psC")
                        else:
                            ps = psC.tile([128, elem], f32, tag="psC")
                        for k, gc in enumerate(gcol_of[w]):
                            first = k == 0
                            last = k == len(gcol_of[w]) - 1
                            Sk = S[:, k * 128:(k + 1) * 128]
                            if layer < 2:
                                nc.tensor.matmul(ps[:], G[:, gc, 0:elem], Sk,
                                                 start=first, stop=False)
                                nc.tensor.matmul(ps[:], G[:, gc, elem:2 * elem], Sk,
                                                 start=False, stop=last)
                            else:
                                nc.tensor.matmul(ps[:], Sk, G[:, gc, 0:elem],
                                                 start=first, stop=False)
                                nc.tensor.matmul(ps[:], Sk, G[:, gc, elem:2 * elem],
                                                 start=False, stop=last)
                        if layer < 2:
                            ndw = wpool.tile([128, 128], f32, tag="ndw")
                            nc.sync.dma_start(ndw[:], ndrep_d[:, w * 128:(w + 1) * 128])
                            t = xpool.tile([128, 128], f32, tag="tagg")
                            nc.vector.tensor_tensor(t[:], ps[:], ndw[:],
                                                    mybir.AluOpType.mult)
                            hsT = wpool.tile([128, 128], f32, tag="hsT")
                            bias = sb0 if layer == 0 else sb1
                            nc.scalar.activation(hsT[:], t[:],
                                                 mybir.ActivationFunctionType.Relu,
                                                 bias=bias[:])
                            if layer == 0:
                                transform_split(w, hsT[:], sW1[:], 128, hp1_own)
                            else:
                                transform_split(w, hsT[:], sW2[:], fcp, hp2_own)
                        else:
                            t = xpool.tile([128, elem], f32, tag="tout")
                            nc.vector.tensor_scalar_mul(t[:], ps[:], sndcol[:, w:w + 1])
                            o = xpool.tile([128, elem], f32, tag="oout")
                            nc.vector.tensor_tensor(o[:], t[:], sb2[:, 0:elem],
                                                    mybir.AluOpType.add)
                            rows = min(128, shard - w * 128)
                            nc.sync.dma_start(out_d[w * 128:w * 128 + rows, :],
                                              o[:rows, 0:f_cls])

            agg_layer(hp0_full, 128, 0)
            nc.gpsimd.collective_compute("AllGather", mybir.AluOpType.bypass, rg,
                                         ins=[hp1_own[:, :]], outs=[hp1_full[:, :]])
            agg_layer(hp1_full, 128, 1)
            nc.gpsimd.collective_compute("AllGather", mybir.AluOpType.bypass, rg,
                                         ins=[hp2_own[:, :]], outs=[hp2_full[:, :]])
            agg_layer(hp2_full, fcp, 2)

    nc.compile()
    return nc


_cache = {}


def kernel(feat, src, dst, W0, b0, W1, b1, W2, b2):
    import ml_dtypes
    feat = np.ascontiguousarray(feat, dtype=np.float32)
    N = feat.shape[0]
    f_cls = np.asarray(W2).shape[1]
    fcp = 64 * ((f_cls + 63) // 64)

    key = (N, hash(np.asarray(src).tobytes()), hash(np.asarray(dst).tobytes()))
    if key in _cache:
        st, per_core, nc_prog = _cache[key]
    else:
        st, per_core = _preprocess(src, dst, N)
        nc_prog = _build_program(st, f_cls)
        _cache[key] = (st, per_core, nc_prog)

    shard, padshard, NW = st['shard'], st['padshard'], st['NW']
    iota = np.tile(np.arange(128, dtype=np.float32), (128, 1))
    W2p = np.zeros((128, fcp), dtype=np.float32)
    W2p[:, :f_cls] = np.asarray(W2, dtype=np.float32)
    b2rep = np.zeros((128, fcp), dtype=np.float32)
    b2rep[:, :f_cls] = np.asarray(b2, dtype=np.float32)[None, :]
    bfv = lambda a: np.ascontiguousarray(a).astype(ml_dtypes.bfloat16)

    in_maps = []
    for c in range(NC):
        pc = per_core[c]
        featT = np.zeros((128, padshard), dtype=np.float32)
        featT[:, :shard] = feat[c * shard:(c + 1) * shard, :].T
        in_maps.append({
            "featT": featT,
            "idx16": pc['idx'],
            "dstl": bfv(pc['dstl']),
            "iota": bfv(iota),
            "ndrep": pc['ndrep'],
            "nscol": pc['nscol'],
            "ndcol": pc['ndcol'],
            "W0": np.asarray(W0, dtype=np.float32),
            "W1": np.asarray(W1, dtype=np.float32),
            "W2p": W2p,
            "b0c": np.asarray(b0, dtype=np.float32).reshape(128, 1),
            "b1c": np.asarray(b1, dtype=np.float32).reshape(128, 1),
            "b2rep": b2rep,
        })

    import os
    trace = os.environ.get("GCN_TRACE") == "1"
    res = run_bass_kernel_spmd(nc_prog, in_maps, core_ids=list(range(NC)),
                               trace=trace)
    global last_results
    last_results = res
    out = np.concatenate([res.results[c]["out"] for c in range(NC)], axis=0)
    return np.ascontiguousarray(out, dtype=np.float32)


last_results = None


# revision 4
# speedup vs baseline: 1.7864x; 1.7864x over previous
"""3-layer GCN (100k nodes, 1.6M edges, 128->128->128->40) on 8 trn2 cores.

Self-contained harness kernel: kernel(**inputs) takes the FULL unsharded
inputs and returns the FULL [100000, 40] float32 output.

Strategy (1D node partition, edges sharded by dst, per the standard GCN
distribution):
  - nodes split contiguously across the 8 cores (12500 each, padded 12544);
    edges assigned to the core owning their dst.
  - per layer, each core computes the dense transform hp = (h @ W) * ns[row]
    for its own rows on the PE (fp32), stores rows as [bf16 hi | bf16 lo]
    (hi/lo split keeps f32-grade accuracy with 1-cycle bf16 matmuls), then an
    AllGather replicates the 100352-row table to every core.
  - aggregation: edges are grouped by 128-wide dst windows; per 128-edge
    chunk a dma_gather pulls the src rows (512B descriptors) and a one-hot
    S matrix (built on DVE from compile-time dst metadata via a broadcast
    is_equal against an iota tile) routes them into a PSUM accumulator via
    two bf16 matmuls (hi+lo).  Window epilogue applies nd / bias / relu on
    DVE+ACT, and the transposed result feeds the next layer's transform as
    the stationary matmul operand directly (no transposes anywhere).
  - gather indices are int16 (table bucketed 4x25088 rows); per-(window,
    bucket) runs are padded to 128 and unioned across cores so all 8 cores
    share one SPMD program.
"""
import sys
sys.path.insert(0, '/opt/trn_rl_repo')

import math
import numpy as np

import concourse.bass as bass
import concourse.bacc as bacc
import concourse.tile as tile
import concourse.mybir as mybir
from concourse.bass_utils import run_bass_kernel_spmd

f32 = mybir.dt.float32
bf16 = mybir.dt.bfloat16
i16 = mybir.dt.int16

NC = 8
GW = 4  # windows per gather group


def _preprocess(src, dst, n_nodes):
    src = np.asarray(src).astype(np.int64)
    dst = np.asarray(dst).astype(np.int64)
    N = n_nodes
    assert N % NC == 0
    shard = N // NC
    NW = (shard + 127) // 128
    padshard = NW * 128
    NPAD = NC * padshard
    NBUK = max(1, math.ceil(NPAD / 32768))
    buksz = math.ceil(NPAD / NBUK / 128) * 128

    outdeg = np.bincount(src, minlength=N)
    indeg = np.bincount(dst, minlength=N)
    ns = (1.0 / np.sqrt(np.maximum(outdeg, 1))).astype(np.float32)
    nd = (1.0 / np.sqrt(np.maximum(indeg, 1))).astype(np.float32)

    srcg = (src // shard) * padshard + (src % shard)
    ecore = dst // shard

    cores = []
    counts = np.zeros((NC, NW, NBUK), dtype=np.int64)
    for c in range(NC):
        m = ecore == c
        es = srcg[m]
        ld = dst[m] - c * shard
        w = ld >> 7
        slot = ld & 127
        b = es // buksz
        reb = es - b * buksz
        order = np.lexsort((es, b, w))
        w, b, slot, reb = w[order], b[order], slot[order], reb[order]
        key = w * NBUK + b
        cnt = np.bincount(key, minlength=NW * NBUK).reshape(NW, NBUK)
        counts[c] = cnt
        cores.append((w, b, slot, reb, key, cnt))

    C_wb = (counts.max(axis=0) + 127) // 128
    C_wb[:, 0] = np.maximum(C_wb[:, 0], 1)
    C_w = C_wb.sum(axis=1)
    TOTCOL = int(C_w.sum())
    TOTSLOT = TOTCOL * 128

    colbase_wb = np.zeros((NW, NBUK), dtype=np.int64)
    acc = 0
    for w_ in range(NW):
        for b_ in range(NBUK):
            colbase_wb[w_, b_] = acc
            acc += C_wb[w_, b_]
    colbase_w = colbase_wb[:, 0]

    groups = []
    idx_off = 0
    NG = (NW + GW - 1) // GW
    for g in range(NG):
        ws = list(range(g * GW, min((g + 1) * GW, NW)))
        calls = []
        slabcol = 0
        for b_ in range(NBUK):
            n_cols = int(C_wb[ws[0]:ws[-1] + 1, b_].sum())
            if n_cols == 0:
                calls.append(None)
                continue
            n_idx = n_cols * 128
            calls.append((b_, idx_off, n_idx, slabcol))
            idx_off += n_idx
            slabcol += n_cols
        groups.append((ws, calls, slabcol))
    assert idx_off == TOTSLOT

    gcol_of = []
    for g, (ws, calls, _) in enumerate(groups):
        for wi, w_ in enumerate(ws):
            lst = []
            for b_ in range(NBUK):
                if calls[b_] is None:
                    continue
                base = calls[b_][3] + int(C_wb[ws[0]:w_, b_].sum())
                for j in range(int(C_wb[w_, b_])):
                    lst.append(base + j)
            gcol_of.append(lst)

    per_core = []
    for c in range(NC):
        w, b, slot, reb, key, cnt = cores[c]
        run_start = np.zeros(NW * NBUK, dtype=np.int64)
        run_start[1:] = np.cumsum(np.bincount(key, minlength=NW * NBUK))[:-1]
        in_run = np.arange(len(key)) - run_start[key]
        base_flat = colbase_wb.reshape(-1) * 128
        pos = base_flat[key] + in_run

        dstl_flat = np.full(TOTSLOT, -1.0, dtype=np.float32)
        dstl_flat[pos] = slot.astype(np.float32)
        idx_flat_winmajor = np.zeros(TOTSLOT, dtype=np.int16)
        idx_flat_winmajor[pos] = reb.astype(np.int16)

        idx_flat = np.zeros(TOTSLOT, dtype=np.int16)
        off = 0
        for g, (ws, calls, _) in enumerate(groups):
            for b_ in range(NBUK):
                if calls[b_] is None:
                    continue
                for w_ in ws:
                    cwb = int(C_wb[w_, b_])
                    if cwb == 0:
                        continue
                    s0 = int(colbase_wb[w_, b_]) * 128
                    n = cwb * 128
                    idx_flat[off:off + n] = idx_flat_winmajor[s0:s0 + n]
                    off += n

        dstl2d = dstl_flat.reshape(TOTCOL, 128).T.copy()
        idx2d = np.tile(idx_flat.reshape(TOTCOL * 8, 16).T, (8, 1)).copy()

        ns_sh = np.zeros(padshard, dtype=np.float32)
        nd_sh = np.zeros(padshard, dtype=np.float32)
        ns_sh[:shard] = ns[c * shard:(c + 1) * shard]
        nd_sh[:shard] = nd[c * shard:(c + 1) * shard]
        nscol = ns_sh.reshape(NW, 128).T.copy()
        ndcol = nd_sh.reshape(NW, 128).T.copy()
        ndrep = np.tile(nd_sh[None, :], (128, 1))

        per_core.append(dict(dstl=dstl2d, idx=idx2d, nscol=nscol, ndcol=ndcol,
                             ndrep=ndrep))

    struct = dict(N=N, shard=shard, NW=NW, padshard=padshard, NPAD=NPAD,
                  NBUK=NBUK, buksz=buksz, C_w=C_w, TOTCOL=TOTCOL,
                  TOTSLOT=TOTSLOT, colbase_w=colbase_w, groups=groups,
                  gcol_of=gcol_of)
    return struct, per_core


def _build_program(st, f_cls):
    NW, padshard, NPAD = st['NW'], st['padshard'], st['NPAD']
    NBUK, buksz = st['NBUK'], st['buksz']
    C_w, TOTCOL, TOTSLOT = st['C_w'], st['TOTCOL'], st['TOTSLOT']
    colbase_w, groups, gcol_of = st['colbase_w'], st['groups'], st['gcol_of']
    shard = st['shard']
    fcp = 64 * ((f_cls + 63) // 64)

    nc = bacc.Bacc(None, target_bir_lowering=False,
                   num_swdge_queues=min(4, NBUK))

    featT_d = nc.dram_tensor("featT", [128, padshard], f32, kind="ExternalInput")
    idx_d = nc.dram_tensor("idx16", [128, TOTSLOT // 16], i16, kind="ExternalInput")
    dstl_d = nc.dram_tensor("dstl", [128, TOTCOL], bf16, kind="ExternalInput")
    iota_d = nc.dram_tensor("iota", [128, 128], bf16, kind="ExternalInput")
    ndrep_d = nc.dram_tensor("ndrep", [128, padshard], f32, kind="ExternalInput")
    nscol_d = nc.dram_tensor("nscol", [128, NW], f32, kind="ExternalInput")
    ndcol_d = nc.dram_tensor("ndcol", [128, NW], f32, kind="ExternalInput")
    W0_d = nc.dram_tensor("W0", [128, 128], f32, kind="ExternalInput")
    W1_d = nc.dram_tensor("W1", [128, 128], f32, kind="ExternalInput")
    W2_d = nc.dram_tensor("W2p", [128, fcp], f32, kind="ExternalInput")
    b0_d = nc.dram_tensor("b0c", [128, 1], f32, kind="ExternalInput")
    b1_d = nc.dram_tensor("b1c", [128, 1], f32, kind="ExternalInput")
    b2_d = nc.dram_tensor("b2rep", [128, fcp], f32, kind="ExternalInput")
    out_d = nc.dram_tensor("out", [shard, f_cls], f32, kind="ExternalOutput")

    hp0_own = nc.dram_tensor("hp0_own", [padshard, 256], bf16)
    hp1_own = nc.dram_tensor("hp1_own", [padshard, 256], bf16)
    hp2_own = nc.dram_tensor("hp2_own", [padshard, 2 * fcp], bf16)
    hp0_full = nc.dram_tensor("hp0_full", [NPAD, 256], bf16, addr_space="Shared")
    hp1_full = nc.dram_tensor("hp1_full", [NPAD, 256], bf16, addr_space="Shared")
    hp2_full = nc.dram_tensor("hp2_full", [NPAD, 2 * fcp], bf16, addr_space="Shared")

    rg = [list(range(NC))]

    with tile.TileContext(nc) as tc:
        with (
            tc.tile_pool(name="const", bufs=1) as cpool,
            tc.tile_pool(name="gpool", bufs=2) as gpool,
            tc.tile_pool(name="spool", bufs=3) as spool,
            tc.tile_pool(name="wpool", bufs=3) as wpool,
            tc.tile_pool(name="xpool", bufs=3) as xpool,
            tc.tile_pool(name="ipool", bufs=2) as ipool,
            tc.tile_pool(name="psA", bufs=2, space="PSUM") as psA,
            tc.tile_pool(name="psC", bufs=4, space="PSUM") as psC,
        ):
            sW0 = cpool.tile([128, 128], f32); nc.sync.dma_start(sW0[:], W0_d[:])
            sW1 = cpool.tile([128, 128], f32); nc.sync.dma_start(sW1[:], W1_d[:])
            sW2 = cpool.tile([128, fcp], f32); nc.sync.dma_start(sW2[:], W2_d[:])
            sb0 = cpool.tile([128, 1], f32); nc.sync.dma_start(sb0[:], b0_d[:])
            sb1 = cpool.tile([128, 1], f32); nc.sync.dma_start(sb1[:], b1_d[:])
            sb2 = cpool.tile([128, fcp], f32); nc.sync.dma_start(sb2[:], b2_d[:])
            siota = cpool.tile([128, 128], bf16); nc.sync.dma_start(siota[:], iota_d[:])
            sdstl = cpool.tile([128, TOTCOL], bf16); nc.sync.dma_start(sdstl[:], dstl_d[:])
            snscol = cpool.tile([128, NW], f32); nc.sync.dma_start(snscol[:], nscol_d[:])
            sndcol = cpool.tile([128, NW], f32); nc.sync.dma_start(sndcol[:], ndcol_d[:])

            def transform_split(w, lhsT_ap, sW, fo, hp_own):
                ps2 = psA.tile([128, fo], f32)
                nc.tensor.matmul(ps2[:], lhsT_ap, sW, start=True, stop=True)
                tns = xpool.tile([128, fo], f32, tag="tns")
                nc.vector.tensor_scalar_mul(tns[:], ps2[:], snscol[:, w:w + 1])
                hp = xpool.tile([128, 2 * fo], bf16, tag="hp")
                nc.scalar.activation(hp[:, 0:fo], tns[:],
                                     mybir.ActivationFunctionType.Copy)
                hif = xpool.tile([128, fo], f32, tag="hif")
                nc.scalar.activation(hif[:], hp[:, 0:fo],
                                     mybir.ActivationFunctionType.Copy)
                nc.vector.tensor_tensor(hp[:, fo:2 * fo], tns[:], hif[:],
                                        mybir.AluOpType.subtract)
                nc.sync.dma_start(hp_own[w * 128:(w + 1) * 128, :], hp[:])

            for w in range(NW):
                ft = wpool.tile([128, 128], f32, tag="ft")
                nc.sync.dma_start(ft[:], featT_d[:, w * 128:(w + 1) * 128])
                transform_split(w, ft[:], sW0[:], 128, hp0_own)
            nc.gpsimd.collective_compute("AllGather", mybir.AluOpType.bypass, rg,
                                         ins=[hp0_own[:, :]], outs=[hp0_full[:, :]])

            def agg_layer(hp_full, elem, layer):
                for (ws, calls, C_g) in groups:
                    G = gpool.tile([128, C_g, 2 * elem], bf16, tag="G")
                    idxs = ipool.tile([128, max(1, C_g) * 8], i16, tag="idx")
                    for call in calls:
                        if call is None:
                            continue
                        b_, off, n_idx, slabcol = call
                        nc.sync.dma_start(
                            idxs[:, slabcol * 8:slabcol * 8 + n_idx // 16],
                            idx_d[:, off // 16:(off + n_idx) // 16])
                        rows = min(buksz, NPAD - b_ * buksz)
                        nc.gpsimd.dma_gather(
                            out_ap=G[:, slabcol:slabcol + n_idx // 128, :],
                            in_ap=hp_full[b_ * buksz:b_ * buksz + rows, :],
                            idxs_ap=idxs[:16, slabcol * 8:slabcol * 8 + n_idx // 16],
                            num_idxs=n_idx,
                            num_idxs_reg=n_idx,
                            elem_size=2 * elem,
                            single_packet=False,
                            queue_num=b_ % min(4, NBUK),
                        )
                    for w in ws:
                        cw = int(C_w[w])
                        cb = int(colbase_w[w])
                        S = spool.tile([128, cw * 128], bf16, tag="S")
                        in0 = sdstl[:, cb:cb + cw].unsqueeze(2).broadcast_to([128, cw, 128])
                        in1 = siota[:, :].unsqueeze(1).broadcast_to([128, cw, 128])
                        nc.vector.tensor_tensor(
                            S[:, :].rearrange("p (c x) -> p c x", x=128),
                            in0, in1, mybir.AluOpType.is_equal)
                        if layer < 2:
                            ps = psC.tile([128, 128], f32, tag="psC")
                        else:
                            ps = psC.tile([128, elem], f32, tag="psC")
                        for k, gc in enumerate(gcol_of[w]):
                            first = k == 0
                            last = k == len(gcol_of[w]) - 1
                            Sk = S[:, k * 128:(k + 1) * 128]
                            if layer < 2:
                                nc.tensor.matmul(ps[:], G[:, gc, 0:elem], Sk,
                                                 start=first, stop=False)
                                nc.tensor.matmul(ps[:], G[:, gc, elem:2 * elem], Sk,
                                                 start=False, stop=last)
                            else:
                                nc.tensor.matmul(ps[:], Sk, G[:, gc, 0:elem],
                                                 start=first, stop=False)
                                nc.tensor.matmul(ps[:], Sk, G[:, gc, elem:2 * elem],
                                                 start=False, stop=last)
                        if layer < 2:
                            ndw = wpool.tile([128, 128], f32, tag="ndw")
                            nc.sync.dma_start(ndw[:], ndrep_d[:, w * 128:(w + 1) * 128])
                            t = xpool.tile([128, 128], f32, tag="tagg")
                            nc.vector.tensor_tensor(t[:], ps[:], ndw[:],
                                                    mybir.AluOpType.mult)
                            hsT = wpool.tile([128, 128], f32, tag="hsT")
                            bias = sb0 if layer == 0 else sb1
                            nc.scalar.activation(hsT[:], t[:],
                                                 mybir.ActivationFunctionType.Relu,
                                                 bias=bias[:])
                            if layer == 0:
                                transform_split(w, hsT[:], sW1[:], 128, hp1_own)
                            else:
                                transform_split(w, hsT[:], sW2[:], fcp, hp2_own)
                        else:
                            t = xpool.tile([128, elem], f32, tag="tout")
                            nc.vector.tensor_scalar_mul(t[:], ps[:], sndcol[:, w:w + 1])
                            o = xpool.tile([128, elem], f32, tag="oout")
                            nc.vector.tensor_tensor(o[:], t[:], sb2[:, 0:elem],
                                                    mybir.AluOpType.add)
                            rows = min(128, shard - w * 128)
                            nc.sync.dma_start(out_d[w * 128:w * 128 + rows, :],
                                              o[:rows, 0:f_cls])

            agg_layer(hp0_full, 128, 0)
            nc.gpsimd.collective_compute("AllGather", mybir.AluOpType.bypass, rg,
                                         ins=[hp1_own[:, :]], outs=[hp1_full[:, :]])
            agg_layer(hp1_full, 128, 1)
            nc.gpsimd.collective_compute("AllGather", mybir.AluOpType.bypass, rg,
                                         ins=[hp2_own[:, :]], outs=[hp2_full[:, :]])
            agg_layer(hp2_full, fcp, 2)

    nc.compile()
    return nc


_cache = {}


def kernel(feat, src, dst, W0, b0, W1, b1, W2, b2):
    import ml_dtypes
    feat = np.ascontiguousarray(feat, dtype=np.float32)
    N = feat.shape[0]
    f_cls = np.asarray(W2).shape[1]
    fcp = 64 * ((f_cls + 63) // 64)

    key = (N, hash(np.asarray(src).tobytes()), hash(np.asarray(dst).tobytes()))
    if key in _cache:
        st, per_core, nc_prog = _cache[key]
    else:
        st, per_core = _preprocess(src, dst, N)
        nc_prog = _build_program(st, f_cls)
        _cache[key] = (st, per_core, nc_prog)

    shard, padshard, NW = st['shard'], st['padshard'], st['NW']
    iota = np.tile(np.arange(128, dtype=np.float32), (128, 1))
    W2p = np.zeros((128, fcp), dtype=np.float32)
    W2p[:, :f_cls] = np.asarray(W2, dtype=np.float32)
    b2rep = np.zeros((128, fcp), dtype=np.float32)
    b2rep[:, :f_cls] = np.asarray(b2, dtype=np.float32)[None, :]
    bfv = lambda a: np.ascontiguousarray(a).astype(ml_dtypes.bfloat16)

    in_maps = []
    for c in range(NC):
        pc = per_core[c]
        featT = np.zeros((128, padshard), dtype=np.float32)
        featT[:, :shard] = feat[c * shard:(c + 1) * shard, :].T
        in_maps.append({
            "featT": featT,
            "idx16": pc['idx'],
            "dstl": bfv(pc['dstl']),
            "iota": bfv(iota),
            "ndrep": pc['ndrep'],
            "nscol": pc['nscol'],
            "ndcol": pc['ndcol'],
            "W0": np.asarray(W0, dtype=np.float32),
            "W1": np.asarray(W1, dtype=np.float32),
            "W2p": W2p,
            "b0c": np.asarray(b0, dtype=np.float32).reshape(128, 1),
            "b1c": np.asarray(b1, dtype=np.float32).reshape(128, 1),
            "b2rep": b2rep,
        })

    import os
    trace = os.environ.get("GCN_TRACE") == "1"
    res = run_bass_kernel_spmd(nc_prog, in_maps, core_ids=list(range(NC)),
                               trace=trace)
    global last_results
    last_results = res
    out = np.concatenate([res.results[c]["out"] for c in range(NC)], axis=0)
    return np.ascontiguousarray(out, dtype=np.float32)


last_results = None


# revision 5
# speedup vs baseline: 1.7968x; 1.0058x over previous
"""3-layer GCN (100k nodes, 1.6M edges, 128->128->128->40) on 8 trn2 cores.

Self-contained harness kernel: kernel(**inputs) takes the FULL unsharded
inputs and returns the FULL [100000, 40] float32 output.

Strategy (1D node partition, edges sharded by dst, per the standard GCN
distribution):
  - nodes split contiguously across the 8 cores (12500 each, padded 12544);
    edges assigned to the core owning their dst.
  - per layer, each core computes the dense transform hp = (h @ W) * ns[row]
    for its own rows on the PE (fp32), stores rows as [bf16 hi | bf16 lo]
    (hi/lo split keeps f32-grade accuracy with 1-cycle bf16 matmuls), then an
    AllGather replicates the 100352-row table to every core.
  - aggregation: edges are grouped by 128-wide dst windows; per 128-edge
    chunk a dma_gather pulls the src rows (512B descriptors) and a one-hot
    S matrix (built on DVE from compile-time dst metadata via a broadcast
    is_equal against an iota tile) routes them into a PSUM accumulator via
    two bf16 matmuls (hi+lo).  Window epilogue applies nd / bias / relu on
    DVE+ACT, and the transposed result feeds the next layer's transform as
    the stationary matmul operand directly (no transposes anywhere).
  - gather indices are int16 (table bucketed 4x25088 rows); per-(window,
    bucket) runs are padded to 128 and unioned across cores so all 8 cores
    share one SPMD program.
"""
import sys
sys.path.insert(0, '/opt/trn_rl_repo')

import math
import numpy as np

import concourse.bass as bass
import concourse.bacc as bacc
import concourse.tile as tile
import concourse.mybir as mybir
from concourse.bass_utils import run_bass_kernel_spmd

f32 = mybir.dt.float32
bf16 = mybir.dt.bfloat16
i16 = mybir.dt.int16

NC = 8
GW = 4  # windows per gather group


def _preprocess(src, dst, n_nodes):
    src = np.asarray(src).astype(np.int64)
    dst = np.asarray(dst).astype(np.int64)
    N = n_nodes
    assert N % NC == 0
    shard = N // NC
    NW = (shard + 127) // 128
    padshard = NW * 128
    NPAD = NC * padshard
    NBUK = max(1, math.ceil(NPAD / 32768))
    buksz = math.ceil(NPAD / NBUK / 128) * 128

    outdeg = np.bincount(src, minlength=N)
    indeg = np.bincount(dst, minlength=N)
    ns = (1.0 / np.sqrt(np.maximum(outdeg, 1))).astype(np.float32)
    nd = (1.0 / np.sqrt(np.maximum(indeg, 1))).astype(np.float32)

    srcg = (src // shard) * padshard + (src % shard)
    ecore = dst // shard

    cores = []
    counts = np.zeros((NC, NW, NBUK), dtype=np.int64)
    for c in range(NC):
        m = ecore == c
        es = srcg[m]
        ld = dst[m] - c * shard
        w = ld >> 7
        slot = ld & 127
        b = es // buksz
        reb = es - b * buksz
        order = np.lexsort((es, b, w))
        w, b, slot, reb = w[order], b[order], slot[order], reb[order]
        key = w * NBUK + b
        cnt = np.bincount(key, minlength=NW * NBUK).reshape(NW, NBUK)
        counts[c] = cnt
        cores.append((w, b, slot, reb, key, cnt))

    C_wb = (counts.max(axis=0) + 127) // 128
    C_wb[:, 0] = np.maximum(C_wb[:, 0], 1)
    C_w = C_wb.sum(axis=1)
    TOTCOL = int(C_w.sum())
    TOTSLOT = TOTCOL * 128

    colbase_wb = np.zeros((NW, NBUK), dtype=np.int64)
    acc = 0
    for w_ in range(NW):
        for b_ in range(NBUK):
            colbase_wb[w_, b_] = acc
            acc += C_wb[w_, b_]
    colbase_w = colbase_wb[:, 0]

    groups = []
    idx_off = 0
    NG = (NW + GW - 1) // GW
    for g in range(NG):
        ws = list(range(g * GW, min((g + 1) * GW, NW)))
        calls = []
        slabcol = 0
        for b_ in range(NBUK):
            n_cols = int(C_wb[ws[0]:ws[-1] + 1, b_].sum())
            if n_cols == 0:
                calls.append(None)
                continue
            n_idx = n_cols * 128
            calls.append((b_, idx_off, n_idx, slabcol))
            idx_off += n_idx
            slabcol += n_cols
        groups.append((ws, calls, slabcol))
    assert idx_off == TOTSLOT

    gcol_of = []
    for g, (ws, calls, _) in enumerate(groups):
        for wi, w_ in enumerate(ws):
            lst = []
            for b_ in range(NBUK):
                if calls[b_] is None:
                    continue
                base = calls[b_][3] + int(C_wb[ws[0]:w_, b_].sum())
                for j in range(int(C_wb[w_, b_])):
                    lst.append(base + j)
            gcol_of.append(lst)

    per_core = []
    for c in range(NC):
        w, b, slot, reb, key, cnt = cores[c]
        run_start = np.zeros(NW * NBUK, dtype=np.int64)
        run_start[1:] = np.cumsum(np.bincount(key, minlength=NW * NBUK))[:-1]
        in_run = np.arange(len(key)) - run_start[key]
        base_flat = colbase_wb.reshape(-1) * 128
        pos = base_flat[key] + in_run

        dstl_flat = np.full(TOTSLOT, -1.0, dtype=np.float32)
        dstl_flat[pos] = slot.astype(np.float32)
        idx_flat_winmajor = np.zeros(TOTSLOT, dtype=np.int16)
        idx_flat_winmajor[pos] = reb.astype(np.int16)

        idx_flat = np.zeros(TOTSLOT, dtype=np.int16)
        off = 0
        for g, (ws, calls, _) in enumerate(groups):
            for b_ in range(NBUK):
                if calls[b_] is None:
                    continue
                for w_ in ws:
                    cwb = int(C_wb[w_, b_])
                    if cwb == 0:
                        continue
                    s0 = int(colbase_wb[w_, b_]) * 128
                    n = cwb * 128
                    idx_flat[off:off + n] = idx_flat_winmajor[s0:s0 + n]
                    off += n

        dstl2d = dstl_flat.reshape(TOTCOL, 128).T.copy()
        idx2d = np.tile(idx_flat.reshape(TOTCOL * 8, 16).T, (8, 1)).copy()

        ns_sh = np.zeros(padshard, dtype=np.float32)
        nd_sh = np.zeros(padshard, dtype=np.float32)
        ns_sh[:shard] = ns[c * shard:(c + 1) * shard]
        nd_sh[:shard] = nd[c * shard:(c + 1) * shard]
        nscol = ns_sh.reshape(NW, 128).T.copy()
        ndcol = nd_sh.reshape(NW, 128).T.copy()
        ndrep = np.tile(nd_sh[None, :], (128, 1))

        per_core.append(dict(dstl=dstl2d, idx=idx2d, nscol=nscol, ndcol=ndcol,
                             ndrep=ndrep))

    struct = dict(N=N, shard=shard, NW=NW, padshard=padshard, NPAD=NPAD,
                  NBUK=NBUK, buksz=buksz, C_w=C_w, TOTCOL=TOTCOL,
                  TOTSLOT=TOTSLOT, colbase_w=colbase_w, groups=groups,
                  gcol_of=gcol_of)
    return struct, per_core


def _build_program(st, f_cls):
    NW, padshard, NPAD = st['NW'], st['padshard'], st['NPAD']
    NBUK, buksz = st['NBUK'], st['buksz']
    C_w, TOTCOL, TOTSLOT = st['C_w'], st['TOTCOL'], st['TOTSLOT']
    colbase_w, groups, gcol_of = st['colbase_w'], st['groups'], st['gcol_of']
    shard = st['shard']
    fcp = 64 * ((f_cls + 63) // 64)

    nc = bacc.Bacc(None, target_bir_lowering=False,
                   num_swdge_queues=min(4, NBUK))

    featT_d = nc.dram_tensor("featT", [128, padshard], f32, kind="ExternalInput")
    idx_d = nc.dram_tensor("idx16", [128, TOTSLOT // 16], i16, kind="ExternalInput")
    dstl_d = nc.dram_tensor("dstl", [128, TOTCOL], bf16, kind="ExternalInput")
    iota_d = nc.dram_tensor("iota", [128, 128], bf16, kind="ExternalInput")
    ndrep_d = nc.dram_tensor("ndrep", [128, padshard], f32, kind="ExternalInput")
    nscol_d = nc.dram_tensor("nscol", [128, NW], f32, kind="ExternalInput")
    ndcol_d = nc.dram_tensor("ndcol", [128, NW], f32, kind="ExternalInput")
    W0_d = nc.dram_tensor("W0", [128, 128], f32, kind="ExternalInput")
    W1_d = nc.dram_tensor("W1", [128, 128], f32, kind="ExternalInput")
    W2_d = nc.dram_tensor("W2p", [128, fcp], f32, kind="ExternalInput")
    b0_d = nc.dram_tensor("b0c", [128, 1], f32, kind="ExternalInput")
    b1_d = nc.dram_tensor("b1c", [128, 1], f32, kind="ExternalInput")
    b2_d = nc.dram_tensor("b2rep", [128, fcp], f32, kind="ExternalInput")
    out_d = nc.dram_tensor("out", [shard, f_cls], f32, kind="ExternalOutput")

    hp0_own = nc.dram_tensor("hp0_own", [padshard, 256], bf16)
    hp1_own = nc.dram_tensor("hp1_own", [padshard, 256], bf16)
    hp2_own = nc.dram_tensor("hp2_own", [padshard, 2 * fcp], bf16)
    hp0_full = nc.dram_tensor("hp0_full", [NPAD, 256], bf16, addr_space="Shared")
    hp1_full = nc.dram_tensor("hp1_full", [NPAD, 256], bf16, addr_space="Shared")
    hp2_full = nc.dram_tensor("hp2_full", [NPAD, 2 * fcp], bf16, addr_space="Shared")

    rg = [list(range(NC))]

    with tile.TileContext(nc) as tc:
        with (
            tc.tile_pool(name="const", bufs=1) as cpool,
            tc.tile_pool(name="gpool", bufs=3) as gpool,
            tc.tile_pool(name="spool", bufs=3) as spool,
            tc.tile_pool(name="wpool", bufs=3) as wpool,
            tc.tile_pool(name="xpool", bufs=3) as xpool,
            tc.tile_pool(name="ipool", bufs=2) as ipool,
            tc.tile_pool(name="psA", bufs=2, space="PSUM") as psA,
            tc.tile_pool(name="psC", bufs=6, space="PSUM") as psC,
        ):
            sW0 = cpool.tile([128, 128], f32); nc.sync.dma_start(sW0[:], W0_d[:])
            sW1 = cpool.tile([128, 128], f32); nc.sync.dma_start(sW1[:], W1_d[:])
            sW2 = cpool.tile([128, fcp], f32); nc.sync.dma_start(sW2[:], W2_d[:])
            sb0 = cpool.tile([128, 1], f32); nc.sync.dma_start(sb0[:], b0_d[:])
            sb1 = cpool.tile([128, 1], f32); nc.sync.dma_start(sb1[:], b1_d[:])
            sb2 = cpool.tile([128, fcp], f32); nc.sync.dma_start(sb2[:], b2_d[:])
            siota = cpool.tile([128, 128], bf16); nc.sync.dma_start(siota[:], iota_d[:])
            sdstl = cpool.tile([128, TOTCOL], bf16); nc.sync.dma_start(sdstl[:], dstl_d[:])
            snscol = cpool.tile([128, NW], f32); nc.sync.dma_start(snscol[:], nscol_d[:])
            sndcol = cpool.tile([128, NW], f32); nc.sync.dma_start(sndcol[:], ndcol_d[:])

            def transform_split(w, lhsT_ap, sW, fo, hp_own):
                ps2 = psA.tile([128, fo], f32)
                nc.tensor.matmul(ps2[:], lhsT_ap, sW, start=True, stop=True)
                tns = xpool.tile([128, fo], f32, tag="tns")
                nc.vector.tensor_scalar_mul(tns[:], ps2[:], snscol[:, w:w + 1])
                hp = xpool.tile([128, 2 * fo], bf16, tag="hp")
                nc.scalar.activation(hp[:, 0:fo], tns[:],
                                     mybir.ActivationFunctionType.Copy)
                hif = xpool.tile([128, fo], f32, tag="hif")
                nc.scalar.activation(hif[:], hp[:, 0:fo],
                                     mybir.ActivationFunctionType.Copy)
                nc.vector.tensor_tensor(hp[:, fo:2 * fo], tns[:], hif[:],
                                        mybir.AluOpType.subtract)
                nc.sync.dma_start(hp_own[w * 128:(w + 1) * 128, :], hp[:])

            for w in range(NW):
                ft = wpool.tile([128, 128], f32, tag="ft")
                nc.scalar.dma_start(ft[:], featT_d[:, w * 128:(w + 1) * 128])
                transform_split(w, ft[:], sW0[:], 128, hp0_own)
            nc.gpsimd.collective_compute("AllGather", mybir.AluOpType.bypass, rg,
                                         ins=[hp0_own[:, :]], outs=[hp0_full[:, :]])

            def agg_layer(hp_full, elem, layer):
                for (ws, calls, C_g) in groups:
                    G = gpool.tile([128, C_g, 2 * elem], bf16, tag="G")
                    idxs = ipool.tile([128, max(1, C_g) * 8], i16, tag="idx")
                    for call in calls:
                        if call is None:
                            continue
                        b_, off, n_idx, slabcol = call
                        nc.sync.dma_start(
                            idxs[:, slabcol * 8:slabcol * 8 + n_idx // 16],
                            idx_d[:, off // 16:(off + n_idx) // 16])
                        rows = min(buksz, NPAD - b_ * buksz)
                        nc.gpsimd.dma_gather(
                            out_ap=G[:, slabcol:slabcol + n_idx // 128, :],
                            in_ap=hp_full[b_ * buksz:b_ * buksz + rows, :],
                            idxs_ap=idxs[:16, slabcol * 8:slabcol * 8 + n_idx // 16],
                            num_idxs=n_idx,
                            num_idxs_reg=n_idx,
                            elem_size=2 * elem,
                            single_packet=False,
                            queue_num=b_ % min(4, NBUK),
                        )
                    for w in ws:
                        cw = int(C_w[w])
                        cb = int(colbase_w[w])
                        S = spool.tile([128, cw * 128], bf16, tag="S")
                        in0 = sdstl[:, cb:cb + cw].unsqueeze(2).broadcast_to([128, cw, 128])
                        in1 = siota[:, :].unsqueeze(1).broadcast_to([128, cw, 128])
                        nc.vector.tensor_tensor(
                            S[:, :].rearrange("p (c x) -> p c x", x=128),
                            in0, in1, mybir.AluOpType.is_equal)
                        if layer < 2:
                            ps = psC.tile([128, 128], f32, tag="psC")
                        else:
                            ps = psC.tile([128, elem], f32, tag="psC")
                        for k, gc in enumerate(gcol_of[w]):
                            first = k == 0
                            last = k == len(gcol_of[w]) - 1
                            Sk = S[:, k * 128:(k + 1) * 128]
                            if layer < 2:
                                nc.tensor.matmul(ps[:], G[:, gc, 0:elem], Sk,
                                                 start=first, stop=False)
                                nc.tensor.matmul(ps[:], G[:, gc, elem:2 * elem], Sk,
                                                 start=False, stop=last)
                            else:
                                nc.tensor.matmul(ps[:], Sk, G[:, gc, 0:elem],
                                                 start=first, stop=False)
                                nc.tensor.matmul(ps[:], Sk, G[:, gc, elem:2 * elem],
                                                 start=False, stop=last)
                        if layer < 2:
                            ndw = wpool.tile([128, 128], f32, tag="ndw")
                            nc.scalar.dma_start(ndw[:], ndrep_d[:, w * 128:(w + 1) * 128])
                            t = xpool.tile([128, 128], f32, tag="tagg")
                            nc.vector.tensor_tensor(t[:], ps[:], ndw[:],
                                                    mybir.AluOpType.mult)
                            hsT = wpool.tile([128, 128], f32, tag="hsT")
                            bias = sb0 if layer == 0 else sb1
                            nc.scalar.activation(hsT[:], t[:],
                                                 mybir.ActivationFunctionType.Relu,
                                                 bias=bias[:])
                            if layer == 0:
                                transform_split(w, hsT[:], sW1[:], 128, hp1_own)
                            else:
                                transform_split(w, hsT[:], sW2[:], fcp, hp2_own)
                        else:
                            t = xpool.tile([128, elem], f32, tag="tout")
                            nc.vector.tensor_scalar_mul(t[:], ps[:], sndcol[:, w:w + 1])
                            o = xpool.tile([128, elem], f32, tag="oout")
                            nc.vector.tensor_tensor(o[:], t[:], sb2[:, 0:elem],
                                                    mybir.AluOpType.add)
                            rows = min(128, shard - w * 128)
                            nc.sync.dma_start(out_d[w * 128:w * 128 + rows, :],
                                              o[:rows, 0:f_cls])

            agg_layer(hp0_full, 128, 0)
            nc.gpsimd.collective_compute("AllGather", mybir.AluOpType.bypass, rg,
                                         ins=[hp1_own[:, :]], outs=[hp1_full[:, :]])
            agg_layer(hp1_full, 128, 1)
            nc.gpsimd.collective_compute("AllGather", mybir.AluOpType.bypass, rg,
                                         ins=[hp2_own[:, :]], outs=[hp2_full[:, :]])
            agg_layer(hp2_full, fcp, 2)

    nc.compile()
    return nc


_cache = {}


def kernel(feat, src, dst, W0, b0, W1, b1, W2, b2):
    import ml_dtypes
    feat = np.ascontiguousarray(feat, dtype=np.float32)
    N = feat.shape[0]
    f_cls = np.asarray(W2).shape[1]
    fcp = 64 * ((f_cls + 63) // 64)

    key = (N, hash(np.asarray(src).tobytes()), hash(np.asarray(dst).tobytes()))
    if key in _cache:
        st, per_core, nc_prog = _cache[key]
    else:
        st, per_core = _preprocess(src, dst, N)
        nc_prog = _build_program(st, f_cls)
        _cache[key] = (st, per_core, nc_prog)

    shard, padshard, NW = st['shard'], st['padshard'], st['NW']
    iota = np.tile(np.arange(128, dtype=np.float32), (128, 1))
    W2p = np.zeros((128, fcp), dtype=np.float32)
    W2p[:, :f_cls] = np.asarray(W2, dtype=np.float32)
    b2rep = np.zeros((128, fcp), dtype=np.float32)
    b2rep[:, :f_cls] = np.asarray(b2, dtype=np.float32)[None, :]
    bfv = lambda a: np.ascontiguousarray(a).astype(ml_dtypes.bfloat16)

    in_maps = []
    for c in range(NC):
        pc = per_core[c]
        featT = np.zeros((128, padshard), dtype=np.float32)
        featT[:, :shard] = feat[c * shard:(c + 1) * shard, :].T
        in_maps.append({
            "featT": featT,
            "idx16": pc['idx'],
            "dstl": bfv(pc['dstl']),
            "iota": bfv(iota),
            "ndrep": pc['ndrep'],
            "nscol": pc['nscol'],
            "ndcol": pc['ndcol'],
            "W0": np.asarray(W0, dtype=np.float32),
            "W1": np.asarray(W1, dtype=np.float32),
            "W2p": W2p,
            "b0c": np.asarray(b0, dtype=np.float32).reshape(128, 1),
            "b1c": np.asarray(b1, dtype=np.float32).reshape(128, 1),
            "b2rep": b2rep,
        })

    import os
    trace = os.environ.get("GCN_TRACE") == "1"
    res = run_bass_kernel_spmd(nc_prog, in_maps, core_ids=list(range(NC)),
                               trace=trace)
    global last_results
    last_results = res
    out = np.concatenate([res.results[c]["out"] for c in range(NC)], axis=0)
    return np.ascontiguousarray(out, dtype=np.float32)


last_results = None
